# revision 25
# baseline (speedup 1.0000x reference)
"""Trainium2 Bass kernel for nn_ConstraintAwareBiasing.

Computes bias[b, n, i, j] = temp[n] * (relu(relu(hi[b,i] + hj[b,j]) @ W2 + b2) @ W3 + b3)[n]
with hi = x @ W1[:128] + b1, hj = x @ W1[128:], masked by `mask`.

Strategy (8 NeuronCores):
  - Shard the (b, i) query axis: core = b*4 + chunk, each core owns 128 i-rows
    against all 512 j for one batch element.
  - Host precomputes hi/hj (tiny [512,128] matmuls), folds head_temperatures
    into W3; b3*temp and the mask are applied on the host.
  - On device, i is processed in 3-row "tiles" for stage 1 and 4-row "groups"
    for stage 2:
      h1 = relu(hjT + hi_col)        tensor_scalar per i; spread over
                                     GpSimd/DVE/ACT (GpSimd cannot touch
                                     PSUM, so s1 is the only compute it can
                                     take - load it up here)
      p1 = W2^T @ h1                 PE matmul into a [128,1536] PSUM tile
                                     (3 i-rows per tile, 3 banks)
      h2 = relu(p1 + b2)             ONE [128,1536] PSUM->SBUF pass per tile
                                     on ACT or DVE (fp32 read is the
                                     bottleneck; fewer+wider is cheaper)
      p2[32c:32c+16] = W3'^T @ h2    PE matmul, col-tiled: 4 i-rows pack into
                                     one PSUM bank; the 4 tiles stream
                                     concurrently (~4ns apart)
      s5: slab[.., g] = cast(p2)     PSUM->SBUF bf16 cast (ACT/DVE)
      per 2 groups: 4 DMAs ship the slab from the GpSimd queue whose DGE
      setup is ~25ns vs ~600ns on sync/scalar.
  - 6 dummy matmuls at t0 keep the PE HAM activity window busy so the clock
    ungates to 2.4 GHz before the real matmul stream.
"""

import numpy as np
import ml_dtypes

import concourse.bass as bass
import concourse.tile as tile
import concourse.mybir as mybir
from concourse import bacc
from concourse.bass_utils import run_bass_kernel_spmd

BF16 = ml_dtypes.bfloat16

B, S, D = 2, 512, 128          # batch, seq, state dim
H, NH = 128, 16                # hidden, heads
N_CORES = 8
CHUNKS = N_CORES // B          # i-chunks per batch element
I_PER_CORE = S // CHUNKS       # 128
GROUPS = I_PER_CORE // 4       # 4 i-rows per group (one PSUM bank of W3 outputs)
NI = 3                         # i-rows per stage-1 PSUM tile (3 banks)
NT = (I_PER_CORE + NI - 1) // NI   # 43 stage-1 tiles (last has 2 rows)
NEG_INF = float("-inf")

_CACHE: dict = {}

# Engine-assignment patterns (tuned against NTFF profiles).
# GpSimd tensor ops measured ~7.5us per [128,512] (software Q7 loop) — never
# assign it compute; it only issues output DMAs.
# s1 per i (i % len): "v" = VectorE, "a" = ScalarE
S1_PAT = ["v"] * 8
# s3 per stage-1 tile (t % len): "a" = ScalarE, "v" = VectorE
# Measured: s3-ACT [128,1536] ~1197ns, s3-DVE ~1799ns, s1 ~263/i,
# s5-DVE ~598, s5-ACT ~569. ALL s3 on ACT (43*1197=51.5us) vs ALL
# s1+s5 on DVE (33.7+19.1=52.8us) balances the engines with no
# head-of-line blocking from mixed assignments.
# Every 8th tile's s3 goes to DVE to relieve the ACT bottleneck; the
# s5 of groups drained on those iterations is forced to ACT so the DVE
# queue lump is bounded at s1x3 + s3v (no triple-stack like plain k=6).
S3_PAT = ["a", "a", "a", "v", "a", "a", "a", "a"]
# s5 per group (g % len); overridden to ACT on v-tile iterations
S5_PAT = ["v"]
# groups per output slab
SLAB_GROUPS = 2
WARM_MMS = 16


def _build_bass():
    nc = bacc.Bacc("TRN2")
    dt = mybir.dt
    hj_d = nc.dram_tensor("hj", (H, S), dt.bfloat16, kind="ExternalInput")
    hi_d = nc.dram_tensor("hi", (H, I_PER_CORE), dt.float32, kind="ExternalInput")
    w2_d = nc.dram_tensor("w2", (H, H), dt.bfloat16, kind="ExternalInput")
    w3_d = nc.dram_tensor("w3", (H, NH), dt.bfloat16, kind="ExternalInput")
    b2_d = nc.dram_tensor("b2", (H, 1), dt.float32, kind="ExternalInput")
    out_d = nc.dram_tensor("out", (I_PER_CORE, NH, S), dt.bfloat16, kind="ExternalOutput")

    relu = mybir.ActivationFunctionType.Relu
    add, amax = mybir.AluOpType.add, mybir.AluOpType.max

    with tile.TileContext(nc) as tc:
        with tc.tile_pool(name="singles", bufs=1) as singles, \
             tc.tile_pool(name="h1p", bufs=10) as h1p, \
             tc.tile_pool(name="h2p", bufs=6) as h2p, \
             tc.tile_pool(name="otp", bufs=3) as otp, \
             tc.tile_pool(name="ps1", bufs=2, space="PSUM") as ps1, \
             tc.tile_pool(name="ps2", bufs=2, space="PSUM") as ps2:
            hj = singles.tile([H, S], dt.bfloat16)
            hi = singles.tile([H, I_PER_CORE], dt.float32)
            w2 = singles.tile([H, H], dt.bfloat16)
            w3 = singles.tile([H, NH], dt.bfloat16)
            b2 = singles.tile([H, 1], dt.float32)
            # input DMAs FIRST so compute can begin ASAP; hj is the
            # critical one (s1 src), spread queues so issues overlap
            nc.sync.dma_start(out=hj[:], in_=hj_d[:])
            nc.scalar.dma_start(out=w2[:], in_=w2_d[:])
            nc.sync.dma_start(out=hi[:], in_=hi_d[:])
            nc.scalar.dma_start(out=w3[:], in_=w3_d[:])
            nc.sync.dma_start(out=b2[:], in_=b2_d[:])
            # PE warmup: dummy matmuls keep the HAM activity window busy
            # so the clock ungates to 2.4 GHz before the real stream
            wrhs = singles.tile([H, S], dt.bfloat16)
            nc.vector.memset(wrhs[:], 0.0)
            wp = ps2.tile([128, S], dt.float32, name="wp", tag="p2")
            for _ in range(8):
                nc.tensor.matmul(wp[:], lhsT=wrhs[:, :H], rhs=wrhs[:],
                                 start=True, stop=True)
            # dummy relu: pulls the ~2.7us ACT table load into the
            # input-DMA wait window instead of serializing at the first s3
            warm = singles.tile([128, 1], dt.float32)
            nc.vector.memset(warm[:], 0.0)
            nc.scalar.activation(out=warm[:], in_=warm[:], func=relu)

            h2tiles = {}      # stage-1 tile index -> h2 SBUF tile
            slab_state = {"tile": None}

            def stage2_w3(g):
                p2 = ps2.tile([128, S], dt.float32, name="p2", tag="p2")
                for c in range(4):
                    i = 4 * g + c
                    tt, off = divmod(i, NI)
                    nc.tensor.matmul(
                        p2[32 * c:32 * c + NH, :], lhsT=w3[:],
                        rhs=h2tiles[tt][:, off * S:(off + 1) * S],
                        start=True, stop=True, tile_position=(0, 32 * c))
                return p2

            def stage2_out(g, p2, s5_eng=None):
                gs = g % SLAB_GROUPS
                if gs == 0:
                    slab_state["tile"] = otp.tile([128, SLAB_GROUPS * S],
                                                  dt.bfloat16, name="ot", tag="ot")
                ot = slab_state["tile"]
                sl = ot[:, gs * S:(gs + 1) * S]
                eng = s5_eng or S5_PAT[g % len(S5_PAT)]
                if eng == "v":
                    nc.vector.tensor_copy(out=sl, in_=p2[:])
                else:
                    nc.scalar.copy(out=sl, in_=p2[:])
                if gs == SLAB_GROUPS - 1:
                    # ship the slab: one DMA per PSUM col group c. src is a
                    # contiguous [16, SLAB_GROUPS*S] partition range; dst
                    # iterates (n, g', j) to match: out_d[i0+c :: 4] is
                    # (g', n, j), so transpose the dram-side AP.
                    i0 = 4 * (g - gs)
                    for c in range(4):
                        dst = out_d[i0 + c:i0 + 4 * SLAB_GROUPS:4]
                        dst = dst.rearrange("g n j -> n g j")
                        eng = nc.gpsimd if c % 2 == 0 else nc.sync
                        eng.dma_start(out=dst, in_=ot[32 * c:32 * c + NH, :])

            done_g = 0
            for t in range(NT + 1):
                # tile t's producer work FIRST (s1 -> mm1 -> s3), then the
                # stage-2 drain for groups covered by tiles < t. This keeps
                # consumer ops (mm2/s5) behind fresh producer ops in each
                # in-order engine queue, so their deps are met by the time
                # they reach the queue head (no head-of-line stalls).
                if t < NT:
                    ni = min(NI, I_PER_CORE - t * NI)
                    q = ps1.tile([H, ni * S], dt.float32, name="q", tag="q")
                    h2t = h2p.tile([H, ni * S], dt.bfloat16, name="h2", tag="h2")
                    h2tiles[t] = h2t
                    for k in range(ni):
                        i = t * NI + k
                        h1 = h1p.tile([H, S], dt.bfloat16)
                        s1e = S1_PAT[i % len(S1_PAT)]
                        if s1e == "a":
                            nc.scalar.activation(
                                out=h1[:], in_=hj[:], func=relu,
                                bias=hi[:, i:i + 1], scale=1.0)
                        else:
                            eng = {"v": nc.vector, "g": nc.gpsimd}[s1e]
                            eng.tensor_scalar(
                                out=h1[:], in0=hj[:], scalar1=hi[:, i:i + 1],
                                scalar2=0.0, op0=add, op1=amax)
                        nc.tensor.matmul(
                            q[:, k * S:(k + 1) * S],
                            lhsT=w2[:], rhs=h1[:], start=True, stop=True)
                    # s3: one wide PSUM->SBUF relu+bias pass for the tile
                    if S3_PAT[t % len(S3_PAT)] == "a":
                        nc.scalar.activation(out=h2t[:], in_=q[:],
                                             func=relu, bias=b2[:], scale=1.0)
                    else:
                        nc.vector.tensor_scalar(
                            out=h2t[:], in0=q[:], scalar1=b2[:, 0:1],
                            scalar2=0.0, op0=add, op1=amax)
                # stage 2 for every group fully covered by tiles < t.
                # On iterations whose tile-t s3 went to DVE, push this
                # iteration's s5 to ACT so the DVE lump stays bounded.
                cur_v = t < NT and S3_PAT[t % len(S3_PAT)] == "v"
                while done_g < GROUPS and (4 * done_g + 3) < (min(t, NT) * NI):
                    g = done_g
                    p2 = stage2_w3(g)
                    stage2_out(g, p2, s5_eng="a" if cur_v else None)
                    done_g += 1
    nc.compile()
    return nc


def _host_prep(inputs):
    x = np.asarray(inputs["state_embeddings"], dtype=np.float32)   # [B, S, D]
    W1 = np.asarray(inputs["W1"], dtype=np.float32)                # [2D, H]
    b1 = np.asarray(inputs["b1"], dtype=np.float32)                # [H]
    W2 = np.asarray(inputs["W2"], dtype=np.float32)                # [H, H]
    b2 = np.asarray(inputs["b2"], dtype=np.float32)                # [H]
    W3 = np.asarray(inputs["W3"], dtype=np.float32)                # [H, NH]
    b3 = np.asarray(inputs["b3"], dtype=np.float32)                # [NH]
    temp = np.asarray(inputs["head_temperatures"], dtype=np.float32)  # [NH]

    hi = x @ W1[:D] + b1                                           # [B, S, H]
    hj = x @ W1[D:]                                                # [B, S, H]
    w3p = (W3 * temp[None, :]).astype(BF16)                        # temp folded in
    b3p = b3 * temp                                                # added on host

    b2col = np.ascontiguousarray(b2.reshape(H, 1))

    in_maps = []
    for core in range(N_CORES):
        b, chunk = divmod(core, CHUNKS)
        i0 = chunk * I_PER_CORE
        in_maps.append({
            "hj": np.ascontiguousarray(hj[b].T).astype(BF16),                  # [H, S]
            "hi": np.ascontiguousarray(hi[b, i0:i0 + I_PER_CORE].T,
                                       dtype=np.float32),                      # [H, I]
            "w2": W2.astype(BF16),
            "w3": w3p,
            "b2": b2col,
        })
    return in_maps, b3p


def _assemble(results, inputs, b3p):
    mask = np.asarray(inputs["mask"])
    out = np.empty((B, NH, S, S), dtype=np.float32)
    for core in range(N_CORES):
        b, chunk = divmod(core, CHUNKS)
        i0 = chunk * I_PER_CORE
        # core result: [I, NH, S] -> out[b, :, i0:i0+I, :]
        out[b, :, i0:i0 + I_PER_CORE, :] = \
            results[core]["out"].transpose(1, 0, 2).astype(np.float32)
    if b3p.any():
        out += b3p[None, :, None, None]
    if not mask.all():
        out = np.where(mask[:, None, :, :], out, np.float32(NEG_INF))
    return out


def _get_nc():
    if "nc" not in _CACHE:
        _CACHE["nc"] = _build_bass()
    return _CACHE["nc"]


def run(inputs, trace=False, **kw):
    nc = _get_nc()
    in_maps, b3p = _host_prep(inputs)
    res = run_bass_kernel_spmd(nc, in_maps, core_ids=list(range(N_CORES)),
                               trace=trace, **kw)
    out = _assemble(res.results, inputs, b3p)
    return out, res


def kernel(**inputs) -> np.ndarray:
    out, _ = run(inputs, trace=False)
    return out



# revision 27
# speedup vs baseline: 1.0005x; 1.0005x over previous
"""Trainium2 Bass kernel for nn_ConstraintAwareBiasing.

Computes bias[b, n, i, j] = temp[n] * (relu(relu(hi[b,i] + hj[b,j]) @ W2 + b2) @ W3 + b3)[n]
with hi = x @ W1[:128] + b1, hj = x @ W1[128:], masked by `mask`.

Strategy (8 NeuronCores):
  - Shard the (b, i) query axis: core = b*4 + chunk, each core owns 128 i-rows
    against all 512 j for one batch element.
  - Host precomputes hi/hj (tiny [512,128] matmuls), folds head_temperatures
    into W3; b3*temp and the mask are applied on the host.
  - On device, i is processed in 3-row "tiles" for stage 1 and 4-row "groups"
    for stage 2:
      h1 = relu(hjT + hi_col)        tensor_scalar per i; spread over
                                     GpSimd/DVE/ACT (GpSimd cannot touch
                                     PSUM, so s1 is the only compute it can
                                     take - load it up here)
      p1 = W2^T @ h1                 PE matmul into a [128,1536] PSUM tile
                                     (3 i-rows per tile, 3 banks)
      h2 = relu(p1 + b2)             ONE [128,1536] PSUM->SBUF pass per tile
                                     on ACT or DVE (fp32 read is the
                                     bottleneck; fewer+wider is cheaper)
      p2[32c:32c+16] = W3'^T @ h2    PE matmul, col-tiled: 4 i-rows pack into
                                     one PSUM bank; the 4 tiles stream
                                     concurrently (~4ns apart)
      s5: slab[.., g] = cast(p2)     PSUM->SBUF bf16 cast (ACT/DVE)
      per 2 groups: 4 DMAs ship the slab from the GpSimd queue whose DGE
      setup is ~25ns vs ~600ns on sync/scalar.
  - 6 dummy matmuls at t0 keep the PE HAM activity window busy so the clock
    ungates to 2.4 GHz before the real matmul stream.
"""

import numpy as np
import ml_dtypes

import concourse.bass as bass
import concourse.tile as tile
import concourse.mybir as mybir
from concourse import bacc
from concourse.bass_utils import run_bass_kernel_spmd

BF16 = ml_dtypes.bfloat16

B, S, D = 2, 512, 128          # batch, seq, state dim
H, NH = 128, 16                # hidden, heads
N_CORES = 8
CHUNKS = N_CORES // B          # i-chunks per batch element
I_PER_CORE = S // CHUNKS       # 128
GROUPS = I_PER_CORE // 4       # 4 i-rows per group (one PSUM bank of W3 outputs)
NI = 3                         # i-rows per stage-1 PSUM tile (3 banks)
NT = (I_PER_CORE + NI - 1) // NI   # 43 stage-1 tiles (last has 2 rows)
NEG_INF = float("-inf")

_CACHE: dict = {}

# Engine-assignment patterns (tuned against NTFF profiles).
# GpSimd tensor ops measured ~7.5us per [128,512] (software Q7 loop) — never
# assign it compute; it only issues output DMAs.
# s1 per i (i % len): "v" = VectorE, "a" = ScalarE
S1_PAT = ["v"] * 8
# s3 per stage-1 tile (t % len): "a" = ScalarE, "v" = VectorE
# Measured: s3-ACT [128,1536] ~1197ns, s3-DVE ~1799ns, s1 ~263/i,
# s5-DVE ~598, s5-ACT ~569. ALL s3 on ACT (43*1197=51.5us) vs ALL
# s1+s5 on DVE (33.7+19.1=52.8us) balances the engines with no
# head-of-line blocking from mixed assignments.
S3_PAT = ["a"]
# s5 per group (g % len); overridden to ACT on v-tile iterations
S5_PAT = ["v"]
# groups per output slab
SLAB_GROUPS = 2
WARM_MMS = 16


def _build_bass():
    nc = bacc.Bacc("TRN2")
    dt = mybir.dt
    hj_d = nc.dram_tensor("hj", (H, S), dt.bfloat16, kind="ExternalInput")
    hi_d = nc.dram_tensor("hi", (H, I_PER_CORE), dt.float32, kind="ExternalInput")
    w2_d = nc.dram_tensor("w2", (H, H), dt.bfloat16, kind="ExternalInput")
    w3_d = nc.dram_tensor("w3", (H, NH), dt.bfloat16, kind="ExternalInput")
    b2_d = nc.dram_tensor("b2", (H, 1), dt.float32, kind="ExternalInput")
    out_d = nc.dram_tensor("out", (I_PER_CORE, NH, S), dt.bfloat16, kind="ExternalOutput")

    relu = mybir.ActivationFunctionType.Relu
    add, amax = mybir.AluOpType.add, mybir.AluOpType.max

    with tile.TileContext(nc) as tc:
        with tc.tile_pool(name="singles", bufs=1) as singles, \
             tc.tile_pool(name="h1p", bufs=10) as h1p, \
             tc.tile_pool(name="h2p", bufs=6) as h2p, \
             tc.tile_pool(name="otp", bufs=3) as otp, \
             tc.tile_pool(name="ps1", bufs=2, space="PSUM") as ps1, \
             tc.tile_pool(name="ps2", bufs=2, space="PSUM") as ps2:
            hj = singles.tile([H, S], dt.bfloat16)
            hi = singles.tile([H, I_PER_CORE], dt.float32)
            w2 = singles.tile([H, H], dt.bfloat16)
            w3 = singles.tile([H, NH], dt.bfloat16)
            b2 = singles.tile([H, 1], dt.float32)
            # input DMAs FIRST so compute can begin ASAP; hj gates the
            # first s1, so split it across both hwdge queues to stream
            # its packets in parallel
            nc.sync.dma_start(out=hj[:, :S // 2], in_=hj_d[:, :S // 2])
            nc.scalar.dma_start(out=hj[:, S // 2:], in_=hj_d[:, S // 2:])
            nc.sync.dma_start(out=hi[:], in_=hi_d[:])
            nc.scalar.dma_start(out=w2[:], in_=w2_d[:])
            nc.scalar.dma_start(out=w3[:], in_=w3_d[:])
            nc.sync.dma_start(out=b2[:], in_=b2_d[:])
            # PE warmup: dummy matmuls keep the HAM activity window busy
            # so the clock ungates to 2.4 GHz before the real stream
            wrhs = singles.tile([H, S], dt.bfloat16)
            nc.vector.memset(wrhs[:], 0.0)
            wp = ps2.tile([128, S], dt.float32, name="wp", tag="p2")
            for _ in range(8):
                nc.tensor.matmul(wp[:], lhsT=wrhs[:, :H], rhs=wrhs[:],
                                 start=True, stop=True)
            # dummy relu: pulls the ~2.7us ACT table load into the
            # input-DMA wait window instead of serializing at the first s3
            warm = singles.tile([128, 1], dt.float32)
            nc.vector.memset(warm[:], 0.0)
            nc.scalar.activation(out=warm[:], in_=warm[:], func=relu)

            h2tiles = {}      # stage-1 tile index -> h2 SBUF tile
            slab_state = {"tile": None}

            def stage2_w3(g):
                p2 = ps2.tile([128, S], dt.float32, name="p2", tag="p2")
                for c in range(4):
                    i = 4 * g + c
                    tt, off = divmod(i, NI)
                    nc.tensor.matmul(
                        p2[32 * c:32 * c + NH, :], lhsT=w3[:],
                        rhs=h2tiles[tt][:, off * S:(off + 1) * S],
                        start=True, stop=True, tile_position=(0, 32 * c))
                return p2

            def stage2_out(g, p2, s5_eng=None):
                gs = g % SLAB_GROUPS
                if gs == 0:
                    slab_state["tile"] = otp.tile([128, SLAB_GROUPS * S],
                                                  dt.bfloat16, name="ot", tag="ot")
                ot = slab_state["tile"]
                sl = ot[:, gs * S:(gs + 1) * S]
                eng = s5_eng or S5_PAT[g % len(S5_PAT)]
                if eng == "v":
                    nc.vector.tensor_copy(out=sl, in_=p2[:])
                else:
                    nc.scalar.copy(out=sl, in_=p2[:])
                if gs == SLAB_GROUPS - 1:
                    # ship the slab: one DMA per PSUM col group c. src is a
                    # contiguous [16, SLAB_GROUPS*S] partition range; dst
                    # iterates (n, g', j) to match: out_d[i0+c :: 4] is
                    # (g', n, j), so transpose the dram-side AP.
                    i0 = 4 * (g - gs)
                    for c in range(4):
                        dst = out_d[i0 + c:i0 + 4 * SLAB_GROUPS:4]
                        dst = dst.rearrange("g n j -> n g j")
                        eng = nc.gpsimd if c % 2 == 0 else nc.sync
                        eng.dma_start(out=dst, in_=ot[32 * c:32 * c + NH, :])

            done_g = 0
            for t in range(NT + 1):
                # tile t's producer work FIRST (s1 -> mm1 -> s3), then the
                # stage-2 drain for groups covered by tiles < t. This keeps
                # consumer ops (mm2/s5) behind fresh producer ops in each
                # in-order engine queue, so their deps are met by the time
                # they reach the queue head (no head-of-line stalls).
                if t < NT:
                    ni = min(NI, I_PER_CORE - t * NI)
                    q = ps1.tile([H, ni * S], dt.float32, name="q", tag="q")
                    h2t = h2p.tile([H, ni * S], dt.bfloat16, name="h2", tag="h2")
                    h2tiles[t] = h2t
                    for k in range(ni):
                        i = t * NI + k
                        h1 = h1p.tile([H, S], dt.bfloat16)
                        s1e = S1_PAT[i % len(S1_PAT)]
                        if s1e == "a":
                            nc.scalar.activation(
                                out=h1[:], in_=hj[:], func=relu,
                                bias=hi[:, i:i + 1], scale=1.0)
                        else:
                            eng = {"v": nc.vector, "g": nc.gpsimd}[s1e]
                            eng.tensor_scalar(
                                out=h1[:], in0=hj[:], scalar1=hi[:, i:i + 1],
                                scalar2=0.0, op0=add, op1=amax)
                        nc.tensor.matmul(
                            q[:, k * S:(k + 1) * S],
                            lhsT=w2[:], rhs=h1[:], start=True, stop=True)
                    # s3: one wide PSUM->SBUF relu+bias pass for the tile
                    if S3_PAT[t % len(S3_PAT)] == "a":
                        nc.scalar.activation(out=h2t[:], in_=q[:],
                                             func=relu, bias=b2[:], scale=1.0)
                    else:
                        nc.vector.tensor_scalar(
                            out=h2t[:], in0=q[:], scalar1=b2[:, 0:1],
                            scalar2=0.0, op0=add, op1=amax)
                # stage 2 for every group fully covered by tiles < t.
                # On iterations whose tile-t s3 went to DVE, push this
                # iteration's s5 to ACT so the DVE lump stays bounded.
                cur_v = t < NT and S3_PAT[t % len(S3_PAT)] == "v"
                while done_g < GROUPS and (4 * done_g + 3) < (min(t, NT) * NI):
                    g = done_g
                    p2 = stage2_w3(g)
                    stage2_out(g, p2, s5_eng="a" if cur_v else None)
                    done_g += 1
    nc.compile()
    return nc


def _host_prep(inputs):
    x = np.asarray(inputs["state_embeddings"], dtype=np.float32)   # [B, S, D]
    W1 = np.asarray(inputs["W1"], dtype=np.float32)                # [2D, H]
    b1 = np.asarray(inputs["b1"], dtype=np.float32)                # [H]
    W2 = np.asarray(inputs["W2"], dtype=np.float32)                # [H, H]
    b2 = np.asarray(inputs["b2"], dtype=np.float32)                # [H]
    W3 = np.asarray(inputs["W3"], dtype=np.float32)                # [H, NH]
    b3 = np.asarray(inputs["b3"], dtype=np.float32)                # [NH]
    temp = np.asarray(inputs["head_temperatures"], dtype=np.float32)  # [NH]

    hi = x @ W1[:D] + b1                                           # [B, S, H]
    hj = x @ W1[D:]                                                # [B, S, H]
    w3p = (W3 * temp[None, :]).astype(BF16)                        # temp folded in
    b3p = b3 * temp                                                # added on host

    b2col = np.ascontiguousarray(b2.reshape(H, 1))

    in_maps = []
    for core in range(N_CORES):
        b, chunk = divmod(core, CHUNKS)
        i0 = chunk * I_PER_CORE
        in_maps.append({
            "hj": np.ascontiguousarray(hj[b].T).astype(BF16),                  # [H, S]
            "hi": np.ascontiguousarray(hi[b, i0:i0 + I_PER_CORE].T,
                                       dtype=np.float32),                      # [H, I]
            "w2": W2.astype(BF16),
            "w3": w3p,
            "b2": b2col,
        })
    return in_maps, b3p


def _assemble(results, inputs, b3p):
    mask = np.asarray(inputs["mask"])
    out = np.empty((B, NH, S, S), dtype=np.float32)
    for core in range(N_CORES):
        b, chunk = divmod(core, CHUNKS)
        i0 = chunk * I_PER_CORE
        # core result: [I, NH, S] -> out[b, :, i0:i0+I, :]
        out[b, :, i0:i0 + I_PER_CORE, :] = \
            results[core]["out"].transpose(1, 0, 2).astype(np.float32)
    if b3p.any():
        out += b3p[None, :, None, None]
    if not mask.all():
        out = np.where(mask[:, None, :, :], out, np.float32(NEG_INF))
    return out


def _get_nc():
    if "nc" not in _CACHE:
        _CACHE["nc"] = _build_bass()
    return _CACHE["nc"]


def run(inputs, trace=False, **kw):
    nc = _get_nc()
    in_maps, b3p = _host_prep(inputs)
    res = run_bass_kernel_spmd(nc, in_maps, core_ids=list(range(N_CORES)),
                               trace=trace, **kw)
    out = _assemble(res.results, inputs, b3p)
    return out, res


def kernel(**inputs) -> np.ndarray:
    out, _ = run(inputs, trace=False)
    return out



# revision 29
# speedup vs baseline: 1.0119x; 1.0114x over previous
"""Trainium2 Bass kernel for nn_ConstraintAwareBiasing.

Computes bias[b, n, i, j] = temp[n] * (relu(relu(hi[b,i] + hj[b,j]) @ W2 + b2) @ W3 + b3)[n]
with hi = x @ W1[:128] + b1, hj = x @ W1[128:], masked by `mask`.

Strategy (8 NeuronCores):
  - Shard the (b, i) query axis: core = b*4 + chunk, each core owns 128 i-rows
    against all 512 j for one batch element.
  - Host precomputes hi/hj (tiny [512,128] matmuls), folds head_temperatures
    into W3; b3*temp and the mask are applied on the host.
  - On device, i is processed in 3-row "tiles" for stage 1 and 4-row "groups"
    for stage 2:
      h1 = relu(hjT + hi_col)        tensor_scalar per i; spread over
                                     GpSimd/DVE/ACT (GpSimd cannot touch
                                     PSUM, so s1 is the only compute it can
                                     take - load it up here)
      p1 = W2^T @ h1                 PE matmul into a [128,1536] PSUM tile
                                     (3 i-rows per tile, 3 banks)
      h2 = relu(p1 + b2)             ONE [128,1536] PSUM->SBUF pass per tile
                                     on ACT or DVE (fp32 read is the
                                     bottleneck; fewer+wider is cheaper)
      p2[32c:32c+16] = W3'^T @ h2    PE matmul, col-tiled: 4 i-rows pack into
                                     one PSUM bank; the 4 tiles stream
                                     concurrently (~4ns apart)
      s5: slab[.., g] = cast(p2)     PSUM->SBUF bf16 cast (ACT/DVE)
      per 2 groups: 4 DMAs ship the slab from the GpSimd queue whose DGE
      setup is ~25ns vs ~600ns on sync/scalar.
  - 6 dummy matmuls at t0 keep the PE HAM activity window busy so the clock
    ungates to 2.4 GHz before the real matmul stream.
"""

import numpy as np
import ml_dtypes

import concourse.bass as bass
import concourse.tile as tile
import concourse.mybir as mybir
from concourse import bacc
from concourse.bass_utils import run_bass_kernel_spmd

BF16 = ml_dtypes.bfloat16

B, S, D = 2, 512, 128          # batch, seq, state dim
H, NH = 128, 16                # hidden, heads
N_CORES = 8
CHUNKS = N_CORES // B          # i-chunks per batch element
I_PER_CORE = S // CHUNKS       # 128
GROUPS = I_PER_CORE // 4       # 4 i-rows per group (one PSUM bank of W3 outputs)
NI = 3                         # i-rows per stage-1 PSUM tile (3 banks)
NT = (I_PER_CORE + NI - 1) // NI   # 43 stage-1 tiles (last has 2 rows)
NEG_INF = float("-inf")

_CACHE: dict = {}

# Engine-assignment patterns (tuned against NTFF profiles).
# GpSimd tensor ops measured ~7.5us per [128,512] (software Q7 loop) — never
# assign it compute; it only issues output DMAs.
# s1 per i (i % len): "v" = VectorE, "a" = ScalarE
S1_PAT = ["v"] * 8
# s3 per stage-1 tile (t % len): "a" = ScalarE, "v" = VectorE
# Measured: s3-ACT [128,1536] ~1197ns, s3-DVE ~1799ns, s1 ~263/i,
# s5-DVE ~598, s5-ACT ~569. ALL s3 on ACT (43*1197=51.5us) vs ALL
# s1+s5 on DVE (33.7+19.1=52.8us) balances the engines with no
# head-of-line blocking from mixed assignments.
S3_PAT = ["a"]
# s5 per group (g % len); overridden to ACT on v-tile iterations
S5_PAT = ["v"]
# groups per output slab
SLAB_GROUPS = 2
WARM_MMS = 16


def _build_bass():
    nc = bacc.Bacc("TRN2")
    dt = mybir.dt
    hj_d = nc.dram_tensor("hj", (H, S), dt.bfloat16, kind="ExternalInput")
    hi_d = nc.dram_tensor("hi", (H, I_PER_CORE), dt.float32, kind="ExternalInput")
    w2_d = nc.dram_tensor("w2", (H, H), dt.bfloat16, kind="ExternalInput")
    w3_d = nc.dram_tensor("w3", (H, NH), dt.bfloat16, kind="ExternalInput")
    b2_d = nc.dram_tensor("b2", (H, 1), dt.float32, kind="ExternalInput")
    out_d = nc.dram_tensor("out", (I_PER_CORE, NH, S), dt.bfloat16, kind="ExternalOutput")

    relu = mybir.ActivationFunctionType.Relu
    add, amax = mybir.AluOpType.add, mybir.AluOpType.max

    with tile.TileContext(nc) as tc:
        with tc.tile_pool(name="singles", bufs=1) as singles, \
             tc.tile_pool(name="h1p", bufs=10) as h1p, \
             tc.tile_pool(name="h2p", bufs=6) as h2p, \
             tc.tile_pool(name="otp", bufs=3) as otp, \
             tc.tile_pool(name="ps1", bufs=2, space="PSUM") as ps1, \
             tc.tile_pool(name="ps2", bufs=2, space="PSUM") as ps2:
            hj = singles.tile([H, S], dt.bfloat16)
            hi = singles.tile([H, I_PER_CORE], dt.float32)
            w2 = singles.tile([H, H], dt.bfloat16)
            w3 = singles.tile([H, NH], dt.bfloat16)
            b2 = singles.tile([H, 1], dt.float32)
            # input DMAs FIRST so compute can begin ASAP; hj gates the
            # first s1, so split it across both hwdge queues to stream
            # its packets in parallel
            nc.sync.dma_start(out=hj[:, :S // 2], in_=hj_d[:, :S // 2])
            nc.scalar.dma_start(out=hj[:, S // 2:], in_=hj_d[:, S // 2:])
            nc.sync.dma_start(out=hi[:], in_=hi_d[:])
            nc.scalar.dma_start(out=w2[:], in_=w2_d[:])
            nc.scalar.dma_start(out=w3[:], in_=w3_d[:])
            nc.sync.dma_start(out=b2[:], in_=b2_d[:])
            # PE warmup: dummy matmuls keep the HAM activity window busy
            # so the clock ungates to 2.4 GHz before the real stream
            wrhs = singles.tile([H, S], dt.bfloat16)
            nc.vector.memset(wrhs[:], 0.0)
            wp = ps2.tile([128, S], dt.float32, name="wp", tag="p2")
            for _ in range(8):
                nc.tensor.matmul(wp[:], lhsT=wrhs[:, :H], rhs=wrhs[:],
                                 start=True, stop=True)
            # dummy relu: pulls the ~2.7us ACT table load into the
            # input-DMA wait window instead of serializing at the first s3
            warm = singles.tile([128, 1], dt.float32)
            nc.vector.memset(warm[:], 0.0)
            nc.scalar.activation(out=warm[:], in_=warm[:], func=relu)

            h2tiles = {}      # stage-1 tile index -> h2 SBUF tile
            imap = {}         # i-row -> (tile index, offset within tile)
            slab_state = {"tile": None}

            def stage2_w3(g):
                p2 = ps2.tile([128, S], dt.float32, name="p2", tag="p2")
                for c in range(4):
                    i = 4 * g + c
                    tt, off = imap[i]
                    nc.tensor.matmul(
                        p2[32 * c:32 * c + NH, :], lhsT=w3[:],
                        rhs=h2tiles[tt][:, off * S:(off + 1) * S],
                        start=True, stop=True, tile_position=(0, 32 * c))
                return p2

            def stage2_out(g, p2):
                # last two groups ship as singleton slabs so the final DMAs
                # fire right after each s5 instead of waiting for a pair
                sz = 1 if g >= GROUPS - 2 else SLAB_GROUPS
                gs = g % SLAB_GROUPS if sz > 1 else 0
                if gs == 0:
                    slab_state["tile"] = otp.tile([128, SLAB_GROUPS * S],
                                                  dt.bfloat16, name="ot", tag="ot")
                ot = slab_state["tile"]
                sl = ot[:, gs * S:(gs + 1) * S]
                if S5_PAT[g % len(S5_PAT)] == "v":
                    nc.vector.tensor_copy(out=sl, in_=p2[:])
                else:
                    nc.scalar.copy(out=sl, in_=p2[:])
                if gs == sz - 1:
                    # ship the slab: one DMA per PSUM col group c. src is a
                    # contiguous [16, sz*S] partition range; dst iterates
                    # (n, g', j) to match: out_d[i0+c :: 4] is (g', n, j),
                    # so transpose the dram-side AP.
                    i0 = 4 * (g - gs)
                    for c in range(4):
                        dst = out_d[i0 + c:i0 + 4 * sz:4]
                        dst = dst.rearrange("g n j -> n g j")
                        eng = nc.gpsimd if c % 2 == 0 else nc.sync
                        eng.dma_start(out=dst, in_=ot[32 * c:32 * c + NH, :sz * S])

            # tile sizes: a small first tile lets ACT's first s3 start
            # ~1us earlier during the fill; the rest stay at NI=3
            nis = [2] + [3] * 42
            assert sum(nis) == I_PER_CORE
            nt = len(nis)
            starts = [0]
            for n_ in nis:
                starts.append(starts[-1] + n_)

            done_g = 0
            for t in range(nt + 1):
                # tile t's producer work FIRST (s1 -> mm1 -> s3), then the
                # stage-2 drain for groups covered by tiles < t. This keeps
                # consumer ops (mm2/s5) behind fresh producer ops in each
                # in-order engine queue, so their deps are met by the time
                # they reach the queue head (no head-of-line stalls).
                if t < nt:
                    ni = nis[t]
                    q = ps1.tile([H, NI * S], dt.float32, name="q", tag="q")
                    h2t = h2p.tile([H, NI * S], dt.bfloat16, name="h2", tag="h2")
                    h2tiles[t] = h2t
                    for k in range(ni):
                        i = starts[t] + k
                        imap[i] = (t, k)
                        h1 = h1p.tile([H, S], dt.bfloat16)
                        s1e = S1_PAT[i % len(S1_PAT)]
                        if s1e == "a":
                            nc.scalar.activation(
                                out=h1[:], in_=hj[:], func=relu,
                                bias=hi[:, i:i + 1], scale=1.0)
                        else:
                            eng = {"v": nc.vector, "g": nc.gpsimd}[s1e]
                            eng.tensor_scalar(
                                out=h1[:], in0=hj[:], scalar1=hi[:, i:i + 1],
                                scalar2=0.0, op0=add, op1=amax)
                        nc.tensor.matmul(
                            q[:, k * S:(k + 1) * S],
                            lhsT=w2[:], rhs=h1[:], start=True, stop=True)
                    # s3: one wide PSUM->SBUF relu+bias pass for the tile
                    if S3_PAT[t % len(S3_PAT)] == "a":
                        nc.scalar.activation(out=h2t[:, :ni * S], in_=q[:, :ni * S],
                                             func=relu, bias=b2[:], scale=1.0)
                    else:
                        nc.vector.tensor_scalar(
                            out=h2t[:, :ni * S], in0=q[:, :ni * S],
                            scalar1=b2[:, 0:1],
                            scalar2=0.0, op0=add, op1=amax)
                # stage 2 for every group fully covered by tiles < t
                while done_g < GROUPS and (4 * done_g + 3) < starts[min(t, nt)]:
                    g = done_g
                    p2 = stage2_w3(g)
                    stage2_out(g, p2)
                    done_g += 1
    nc.compile()
    return nc


def _host_prep(inputs):
    x = np.asarray(inputs["state_embeddings"], dtype=np.float32)   # [B, S, D]
    W1 = np.asarray(inputs["W1"], dtype=np.float32)                # [2D, H]
    b1 = np.asarray(inputs["b1"], dtype=np.float32)                # [H]
    W2 = np.asarray(inputs["W2"], dtype=np.float32)                # [H, H]
    b2 = np.asarray(inputs["b2"], dtype=np.float32)                # [H]
    W3 = np.asarray(inputs["W3"], dtype=np.float32)                # [H, NH]
    b3 = np.asarray(inputs["b3"], dtype=np.float32)                # [NH]
    temp = np.asarray(inputs["head_temperatures"], dtype=np.float32)  # [NH]

    hi = x @ W1[:D] + b1                                           # [B, S, H]
    hj = x @ W1[D:]                                                # [B, S, H]
    w3p = (W3 * temp[None, :]).astype(BF16)                        # temp folded in
    b3p = b3 * temp                                                # added on host

    b2col = np.ascontiguousarray(b2.reshape(H, 1))

    in_maps = []
    for core in range(N_CORES):
        b, chunk = divmod(core, CHUNKS)
        i0 = chunk * I_PER_CORE
        in_maps.append({
            "hj": np.ascontiguousarray(hj[b].T).astype(BF16),                  # [H, S]
            "hi": np.ascontiguousarray(hi[b, i0:i0 + I_PER_CORE].T,
                                       dtype=np.float32),                      # [H, I]
            "w2": W2.astype(BF16),
            "w3": w3p,
            "b2": b2col,
        })
    return in_maps, b3p


def _assemble(results, inputs, b3p):
    mask = np.asarray(inputs["mask"])
    out = np.empty((B, NH, S, S), dtype=np.float32)
    for core in range(N_CORES):
        b, chunk = divmod(core, CHUNKS)
        i0 = chunk * I_PER_CORE
        # core result: [I, NH, S] -> out[b, :, i0:i0+I, :]
        out[b, :, i0:i0 + I_PER_CORE, :] = \
            results[core]["out"].transpose(1, 0, 2).astype(np.float32)
    if b3p.any():
        out += b3p[None, :, None, None]
    if not mask.all():
        out = np.where(mask[:, None, :, :], out, np.float32(NEG_INF))
    return out


def _get_nc():
    if "nc" not in _CACHE:
        _CACHE["nc"] = _build_bass()
    return _CACHE["nc"]


def run(inputs, trace=False, **kw):
    nc = _get_nc()
    in_maps, b3p = _host_prep(inputs)
    res = run_bass_kernel_spmd(nc, in_maps, core_ids=list(range(N_CORES)),
                               trace=trace, **kw)
    out = _assemble(res.results, inputs, b3p)
    return out, res


def kernel(**inputs) -> np.ndarray:
    out, _ = run(inputs, trace=False)
    return out

